# revision 14
# baseline (speedup 1.0000x reference)
"""Trainium2 Bass kernel for KNN-upsample (MLP on down points + KNN mean + residual).

Contract: kernel(**inputs) takes FULL numpy inputs (as produced by
setup_inputs) and returns the FULL (LU, N, D_OUT) float32 output.

Sharding: batch axis N=8 -> one NeuronCore per batch column (data
parallel, no cross-core communication).

Per-core device program:
  Phase A (MLP, bf16): h2[m, :] = relu(down[m, :] @ W1 + b1) @ (W2/3),
    computed per 512-token PSUM stripe into a persistent SBUF table
    h2sb[p, r, f] = h2[r*128+p, f] (bf16, 32 KB/partition), which is
    also staged out to a DRAM copy h2x (256 B rows).
  Phase B (gather + combine): the point space is cut into chunks of
    C=1024.  Three of every four chunks gather their K*C rows from the
    DRAM copy with the plain CME dma_gather on SWDGE queues 0..2 -
    descriptor generation runs on three Q7 core pairs concurrently.
    Every fourth chunk instead uses the transposing SBUF-source gather
    on queue 3 (the fourth core pair), which reads h2sb over the SBUF
    fabric and so takes no HBM random-read bandwidth.  The transpose
    stream is serialized with itself (one queue): overlapping transpose
    gathers corrupt each other through the shared XBAR, but one
    transpose stream + CME streams verify bit-exact.
    CME chunks combine token-major (slot j = (k*upc+u)*128+p), transpose
    chunks feature-major (slot j = k*C+i); DVE sums the three k-planes
    plus the bias-folded up features; results DMA out in bf16 and the
    host upcasts.

Host-side preprocessing (pure data-layout / index work):
  - down column transposed to (D_IN, LD), cast bf16; W1, W2/3 bf16
  - b2 folded into up; up packed per chunk type (token-major
    [cc, p, u, f] / feature-major [ct, f, i]) in bf16
  - idx cast int16, slot-ordered per chunk type, wrapped into the
    [16, ni/16] SWDGE layout, replicated x8, all chunks concatenated
  - outputs unpacked per chunk type and upcast to fp32
"""

import numpy as np
import ml_dtypes
from contextlib import ExitStack

import concourse.bacc as bacc
import concourse.tile as tile
import concourse.mybir as mybir
from concourse.bass_utils import run_bass_kernel_spmd

LD, LU, N, D_IN, D_OUT, K = 16384, 65536, 8, 256, 128, 3

CHUNK = 1024            # upsample points per gather chunk
MLP_BLOCK = 2048        # down points per MLP dma block
NCORES = 8
TR_EVERY = 4            # every TR_EVERY-th chunk uses the SBUF transpose path

F32 = mybir.dt.float32
BF16 = mybir.dt.bfloat16
I16 = mybir.dt.int16

_BUILD_CACHE = {}


def _chunk_types(nchunk):
    """Returns (is_tr list, cme order index, tr order index, counts)."""
    is_tr, cme_ix, tr_ix = [], [], []
    nc_, nt_ = 0, 0
    for c in range(nchunk):
        t = (c % TR_EVERY == TR_EVERY - 1)
        is_tr.append(t)
        cme_ix.append(None if t else nc_)
        tr_ix.append(nt_ if t else None)
        nc_ += 0 if t else 1
        nt_ += 1 if t else 0
    return is_tr, cme_ix, tr_ix, nc_, nt_


def _build(ld=LD, lu=LU, d_in=D_IN, d_out=D_OUT, chunk=CHUNK, mlp_block=MLP_BLOCK):
    """Build + compile the per-core Bass program (identical on all cores)."""
    key = (ld, lu, d_in, d_out, chunk, mlp_block)
    if key in _BUILD_CACHE:
        return _BUILD_CACHE[key]

    nchunk = lu // chunk
    upc = chunk // 128                  # 128-point groups per chunk
    ni = K * chunk                      # gather indices per chunk
    nkb = d_in // 128                   # contraction tiles for matmul 1
    nblk = ld // mlp_block              # MLP dma blocks
    spb = mlp_block // 128              # 128-token sub-tiles per MLP block
    nrank = ld // 128

    is_tr, cme_ix, tr_ix, n_cme, n_tr = _chunk_types(nchunk)

    nc = bacc.Bacc("TRN2", target_bir_lowering=False, debug=False,
                   num_swdge_queues=4)

    downt_d = nc.dram_tensor("downt", (d_in, ld), BF16, kind="ExternalInput")
    w1_d = nc.dram_tensor("w1", (d_in, d_out), BF16, kind="ExternalInput")
    b1_d = nc.dram_tensor("b1", (d_out, 1), F32, kind="ExternalInput")
    w2s_d = nc.dram_tensor("w2s", (d_out, d_out), BF16, kind="ExternalInput")
    upc_d = nc.dram_tensor("upcme", (max(n_cme, 1), 128, upc, d_out), BF16,
                           kind="ExternalInput")
    uptr_d = nc.dram_tensor("uptr", (max(n_tr, 1), 128, chunk), BF16,
                            kind="ExternalInput")
    idx_d = nc.dram_tensor("idxp", (128, nchunk * (ni // 16)), I16,
                           kind="ExternalInput")
    outc_d = nc.dram_tensor("outc", (max(n_cme, 1), 128, upc, d_out), BF16,
                            kind="ExternalOutput")
    outt_d = nc.dram_tensor("outt", (max(n_tr, 1), 128, chunk), BF16,
                            kind="ExternalOutput")
    h2x_d = nc.dram_tensor("h2x", (ld, d_out), BF16, kind="Internal")

    relu = mybir.ActivationFunctionType.Relu

    # h2x viewed so a whole MLP block stores with one DMA:
    # row index m = b*mlp_block + s*128 + p  ->  [b, p, s, f]
    h2x_blk = h2x_d.ap().rearrange("(b s p) f -> b p s f", b=nblk, s=spb, p=128)

    with tile.TileContext(nc) as tc, ExitStack() as ctx:
        consts = ctx.enter_context(tc.tile_pool(name="consts", bufs=1))
        ix_all = consts.tile([128, nchunk * (ni // 16)], I16)
        w1_t = consts.tile([128, nkb, d_out], BF16)
        w1_v = w1_d.ap().rearrange("(j p) e -> j p e", j=nkb, p=128)
        for j in range(nkb):
            nc.sync.dma_start(w1_t[:, j, :], w1_v[j])
        w2s_t = consts.tile([128, d_out], BF16)
        nc.sync.dma_start(w2s_t[:], w2s_d.ap())
        b1_t = consts.tile([128, 1], F32)
        nc.sync.dma_start(b1_t[:], b1_d.ap())
        # persistent token-major h2 table: [p, r, f] = h2[r*128+p, f]
        h2sb = consts.tile([128, nrank, d_out], BF16)

        # ---------------- Phase A: MLP ----------------
        stripe = 512                      # PSUM bank / max moving free dim
        with tc.tile_pool(name="dn", bufs=4) as dn_pool, \
             tc.tile_pool(name="ps1", bufs=3, space="PSUM") as ps1_pool, \
             tc.tile_pool(name="hT", bufs=3) as hT_pool, \
             tc.tile_pool(name="ps2", bufs=4, space="PSUM") as ps2_pool:
            for b in range(nblk):
                dn = dn_pool.tile([128, nkb, mlp_block], BF16)
                for j in range(nkb):
                    nc.sync.dma_start(
                        dn[:, j, :],
                        downt_d.ap()[j * 128:(j + 1) * 128,
                                     b * mlp_block:(b + 1) * mlp_block])
                if b == 1:
                    nc.sync.dma_start(ix_all[:], idx_d.ap())
                hT = hT_pool.tile([128, mlp_block], BF16)
                for t0 in range(0, mlp_block, stripe):
                    ps1 = ps1_pool.tile([128, stripe], F32)
                    for j in range(nkb):
                        nc.tensor.matmul(ps1[:], w1_t[:, j, :],
                                         dn[:, j, t0:t0 + stripe],
                                         start=(j == 0), stop=(j == nkb - 1))
                    nc.scalar.activation(hT[:, t0:t0 + stripe], ps1[:], relu,
                                         bias=b1_t[:])
                for s in range(spb):
                    ps2 = ps2_pool.tile([128, d_out], F32)
                    nc.tensor.matmul(ps2[:], hT[:, s * 128:(s + 1) * 128],
                                     w2s_t[:], start=True, stop=True)
                    nc.vector.tensor_copy(h2sb[:, b * spb + s, :], ps2[:])
                nc.sync.dma_start(h2x_blk[b],
                                  h2sb[:, b * spb:(b + 1) * spb, :])

        # ---------------- Phase B: gather + combine ----------------
        cme_q = 0
        with tc.tile_pool(name="gat", bufs=10) as g_pool, \
             tc.tile_pool(name="upt", bufs=4) as up_pool, \
             tc.tile_pool(name="tt", bufs=4) as t_pool, \
             tc.tile_pool(name="ot", bufs=4) as o_pool:
            for c in range(nchunk):
                sl = ix_all[:, c * (ni // 16):(c + 1) * (ni // 16)]
                g = g_pool.tile([128, K * chunk], BF16)
                upt = up_pool.tile([128, chunk], BF16)
                t = t_pool.tile([128, chunk], F32)
                o = o_pool.tile([128, chunk], BF16)
                if is_tr[c]:
                    # feature-major SBUF-source transpose gather, queue 3
                    nc.gpsimd.dma_gather(
                        g[:].rearrange("p (o n) -> p o n", o=1), h2sb[:], sl,
                        num_idxs=ni, num_idxs_reg=ni, elem_size=d_out,
                        transpose=True, single_packet=False, queue_num=3,
                        sbuf_tokens_per_rank=128,
                        sbuf_free_dim_per_rank=d_out * 2,
                        sbuf_free_dim_pad_per_rank=0, sbuf_byte_offset=0)
                    nc.sync.dma_start(upt[:], uptr_d.ap()[tr_ix[c]])
                    nc.vector.tensor_add(t[:], g[:, 0:chunk],
                                         g[:, chunk:2 * chunk])
                    nc.vector.tensor_add(t[:], t[:],
                                         g[:, 2 * chunk:3 * chunk])
                    nc.vector.tensor_add(o[:], t[:], upt[:])
                    nc.sync.dma_start(outt_d.ap()[tr_ix[c]], o[:])
                else:
                    # token-major CME gather from DRAM, queues 0..2
                    gv = g[:].rearrange("p (k u f) -> p (k u) f", k=K, u=upc)
                    nc.gpsimd.dma_gather(
                        gv, h2x_d.ap(), sl,
                        num_idxs=ni, num_idxs_reg=ni, elem_size=d_out,
                        single_packet=False, queue_num=cme_q)
                    cme_q = (cme_q + 1) % 3
                    g4 = g[:].rearrange("p (k u f) -> p k (u f)", k=K, u=upc)
                    nc.sync.dma_start(
                        upt[:].rearrange("p (u f) -> p u f", u=upc),
                        upc_d.ap()[cme_ix[c]])
                    nc.vector.tensor_add(t[:], g4[:, 0], g4[:, 1])
                    nc.vector.tensor_add(t[:], t[:], g4[:, 2])
                    nc.vector.tensor_add(o[:], t[:], upt[:])
                    nc.sync.dma_start(
                        outc_d.ap()[cme_ix[c]],
                        o[:].rearrange("p (u f) -> p u f", u=upc))

    nc.compile()
    _BUILD_CACHE[key] = nc
    return nc


def _prep_core_inputs(down_features, up_features, idx, W1, b1, W2, b2, n,
                      ld=LD, lu=LU, d_in=D_IN, d_out=D_OUT, chunk=CHUNK):
    """Host-side packing of the full inputs into core n's input map."""
    nchunk = lu // chunk
    upc = chunk // 128
    ni = K * chunk
    is_tr, cme_ix, tr_ix, n_cme, n_tr = _chunk_types(nchunk)

    downt = np.ascontiguousarray(
        down_features[:, n, :].T).astype(ml_dtypes.bfloat16)
    upn = up_features[:, n, :].astype(np.float32) + b2[None, :].astype(np.float32)
    upn = upn.reshape(nchunk, chunk, d_out)
    idxn = idx[:, n, :].astype(np.int16).reshape(nchunk, chunk, K)

    upcme = np.zeros((max(n_cme, 1), 128, upc, d_out), np.float32)
    uptr = np.zeros((max(n_tr, 1), 128, chunk), np.float32)
    flat = np.empty((nchunk, ni), np.int16)
    for c in range(nchunk):
        if is_tr[c]:
            # slots j = k*chunk + i; feature-major up
            flat[c] = idxn[c].T.reshape(ni)                    # [k, i]
            uptr[tr_ix[c]] = upn[c].T
        else:
            # slots j = (k*upc + u)*128 + p; token-major chunk layout
            flat[c] = idxn[c].reshape(upc, 128, K).transpose(2, 0, 1).reshape(ni)
            upcme[cme_ix[c]] = upn[c].reshape(upc, 128, d_out).transpose(1, 0, 2)

    wrapped = flat.reshape(nchunk, ni // 16, 16).transpose(0, 2, 1)
    rep = np.tile(wrapped, (1, 8, 1))                          # [c, 128, ni/16]
    idxp = np.ascontiguousarray(
        rep.transpose(1, 0, 2).reshape(128, nchunk * (ni // 16)))

    return {
        "downt": downt,
        "w1": np.ascontiguousarray(W1).astype(ml_dtypes.bfloat16),
        "b1": np.ascontiguousarray(b1.astype(np.float32).reshape(d_out, 1)),
        "w2s": np.ascontiguousarray(W2.astype(np.float32)
                                    / np.float32(K)).astype(ml_dtypes.bfloat16),
        "upcme": upcme.astype(ml_dtypes.bfloat16),
        "uptr": uptr.astype(ml_dtypes.bfloat16),
        "idxp": idxp,
    }


def _unpack_out(res_map, lu=LU, d_out=D_OUT, chunk=CHUNK):
    nchunk = lu // chunk
    upc = chunk // 128
    is_tr, cme_ix, tr_ix, n_cme, n_tr = _chunk_types(nchunk)
    outc = np.asarray(res_map["outc"]).astype(np.float32)
    outt = np.asarray(res_map["outt"]).astype(np.float32)
    out = np.empty((lu, d_out), np.float32)
    for c in range(nchunk):
        dst = out[c * chunk:(c + 1) * chunk]
        if is_tr[c]:
            dst[:] = outt[tr_ix[c]].T
        else:
            dst[:] = outc[cme_ix[c]].transpose(1, 0, 2).reshape(chunk, d_out)
    return out


def kernel(down_features, up_features, idx, W1, b1, W2, b2):
    down_features = np.asarray(down_features)
    up_features = np.asarray(up_features)
    idx = np.asarray(idx)
    W1, b1, W2, b2 = (np.asarray(a) for a in (W1, b1, W2, b2))

    nc = _build()
    in_maps = [
        _prep_core_inputs(down_features, up_features, idx, W1, b1, W2, b2, n)
        for n in range(NCORES)
    ]
    res = run_bass_kernel_spmd(nc, in_maps, core_ids=list(range(NCORES)))
    cols = [_unpack_out(res.results[n]) for n in range(NCORES)]
    return np.stack(cols, axis=1).astype(np.float32)


# revision 15
# speedup vs baseline: 1.2039x; 1.2039x over previous
"""Trainium2 Bass kernel for KNN-upsample (MLP on down points + KNN mean + residual).

Contract: kernel(**inputs) takes FULL numpy inputs (as produced by
setup_inputs) and returns the FULL (LU, N, D_OUT) float32 output.

Sharding: batch axis N=8 -> one NeuronCore per batch column (data
parallel, no cross-core communication).

Per-core device program:
  Phase A (MLP, bf16): h2[m, :] = relu(down[m, :] @ W1 + b1) @ (W2/3),
    computed per 512-token PSUM stripe into a persistent SBUF table
    h2sb[p, r, f] = h2[r*128+p, f] (bf16, 32 KB/partition), which is
    also staged out to a DRAM copy h2x (256 B rows).
  Phase B (gather + combine): the point space is cut into chunks of
    C=1024.  Three of every four chunks gather their K*C rows from the
    DRAM copy with the plain CME dma_gather on SWDGE queues 0..2 -
    descriptor generation runs on three Q7 core pairs concurrently.
    Every fourth chunk instead uses the transposing SBUF-source gather
    on queue 3 (the fourth core pair), which reads h2sb over the SBUF
    fabric and so takes no HBM random-read bandwidth.  The transpose
    stream is serialized with itself (one queue): overlapping transpose
    gathers corrupt each other through the shared XBAR, but one
    transpose stream + CME streams verify bit-exact.
    CME chunks combine token-major (slot j = (k*upc+u)*128+p), transpose
    chunks feature-major (slot j = k*C+i); DVE sums the three k-planes
    plus the bias-folded up features; results DMA out in bf16 and the
    host upcasts.

Host-side preprocessing (pure data-layout / index work):
  - down column transposed to (D_IN, LD), cast bf16; W1, W2/3 bf16
  - b2 folded into up; up packed per chunk type (token-major
    [cc, p, u, f] / feature-major [ct, f, i]) in bf16
  - idx cast int16, slot-ordered per chunk type, wrapped into the
    [16, ni/16] SWDGE layout, replicated x8, all chunks concatenated
  - outputs unpacked per chunk type and upcast to fp32
"""

import numpy as np
import ml_dtypes
from contextlib import ExitStack

import concourse.bacc as bacc
import concourse.tile as tile
import concourse.mybir as mybir
from concourse.bass_utils import run_bass_kernel_spmd

LD, LU, N, D_IN, D_OUT, K = 16384, 65536, 8, 256, 128, 3

CHUNK = 1024            # upsample points per gather chunk
MLP_BLOCK = 2048        # down points per MLP dma block
NCORES = 8
TR_EVERY = 4            # every TR_EVERY-th chunk uses the SBUF transpose path

F32 = mybir.dt.float32
BF16 = mybir.dt.bfloat16
I16 = mybir.dt.int16

_BUILD_CACHE = {}


def _chunk_types(nchunk):
    """Returns (is_tr list, cme order index, tr order index, counts)."""
    is_tr, cme_ix, tr_ix = [], [], []
    nc_, nt_ = 0, 0
    for c in range(nchunk):
        t = (c % TR_EVERY == TR_EVERY - 1)
        is_tr.append(t)
        cme_ix.append(None if t else nc_)
        tr_ix.append(nt_ if t else None)
        nc_ += 0 if t else 1
        nt_ += 1 if t else 0
    return is_tr, cme_ix, tr_ix, nc_, nt_


def _build(ld=LD, lu=LU, d_in=D_IN, d_out=D_OUT, chunk=CHUNK, mlp_block=MLP_BLOCK):
    """Build + compile the per-core Bass program (identical on all cores)."""
    key = (ld, lu, d_in, d_out, chunk, mlp_block)
    if key in _BUILD_CACHE:
        return _BUILD_CACHE[key]

    nchunk = lu // chunk
    upc = chunk // 128                  # 128-point groups per chunk
    ni = K * chunk                      # gather indices per chunk
    nkb = d_in // 128                   # contraction tiles for matmul 1
    nblk = ld // mlp_block              # MLP dma blocks
    spb = mlp_block // 128              # 128-token sub-tiles per MLP block
    nrank = ld // 128

    is_tr, cme_ix, tr_ix, n_cme, n_tr = _chunk_types(nchunk)

    nc = bacc.Bacc("TRN2", target_bir_lowering=False, debug=False,
                   num_swdge_queues=4)

    downt_d = nc.dram_tensor("downt", (d_in, ld), BF16, kind="ExternalInput")
    w1_d = nc.dram_tensor("w1", (d_in, d_out), BF16, kind="ExternalInput")
    b1_d = nc.dram_tensor("b1", (d_out, 1), F32, kind="ExternalInput")
    w2s_d = nc.dram_tensor("w2s", (d_out, d_out), BF16, kind="ExternalInput")
    upc_d = nc.dram_tensor("upcme", (max(n_cme, 1), 128, upc, d_out), BF16,
                           kind="ExternalInput")
    uptr_d = nc.dram_tensor("uptr", (max(n_tr, 1), 128, chunk), BF16,
                            kind="ExternalInput")
    idx_d = nc.dram_tensor("idxp", (128, nchunk * (ni // 16)), I16,
                           kind="ExternalInput")
    outc_d = nc.dram_tensor("outc", (max(n_cme, 1), 128, upc, d_out), BF16,
                            kind="ExternalOutput")
    outt_d = nc.dram_tensor("outt", (max(n_tr, 1), 128, chunk), BF16,
                            kind="ExternalOutput")
    h2x_d = nc.dram_tensor("h2x", (ld, d_out), BF16, kind="Internal")

    relu = mybir.ActivationFunctionType.Relu

    # h2x viewed so a whole MLP block stores with one DMA:
    # row index m = b*mlp_block + s*128 + p  ->  [b, p, s, f]
    h2x_blk = h2x_d.ap().rearrange("(b s p) f -> b p s f", b=nblk, s=spb, p=128)

    with tile.TileContext(nc) as tc, ExitStack() as ctx:
        consts = ctx.enter_context(tc.tile_pool(name="consts", bufs=1))
        ix_all = consts.tile([128, nchunk * (ni // 16)], I16)
        w1_t = consts.tile([128, nkb, d_out], BF16)
        w1_v = w1_d.ap().rearrange("(j p) e -> j p e", j=nkb, p=128)
        for j in range(nkb):
            nc.sync.dma_start(w1_t[:, j, :], w1_v[j])
        w2s_t = consts.tile([128, d_out], BF16)
        nc.sync.dma_start(w2s_t[:], w2s_d.ap())
        b1_t = consts.tile([128, 1], F32)
        nc.sync.dma_start(b1_t[:], b1_d.ap())
        # persistent token-major h2 table: [p, r, f] = h2[r*128+p, f]
        h2sb = consts.tile([128, nrank, d_out], BF16)

        # ---------------- Phase A: MLP ----------------
        stripe = 512                      # PSUM bank / max moving free dim
        with tc.tile_pool(name="dn", bufs=4) as dn_pool, \
             tc.tile_pool(name="ps1", bufs=3, space="PSUM") as ps1_pool, \
             tc.tile_pool(name="hT", bufs=3) as hT_pool, \
             tc.tile_pool(name="ps2", bufs=4, space="PSUM") as ps2_pool:
            for b in range(nblk):
                dn = dn_pool.tile([128, nkb, mlp_block], BF16)
                for j in range(nkb):
                    nc.sync.dma_start(
                        dn[:, j, :],
                        downt_d.ap()[j * 128:(j + 1) * 128,
                                     b * mlp_block:(b + 1) * mlp_block])
                ixs = ix_all.shape[1] // nblk
                nc.sync.dma_start(ix_all[:, b * ixs:(b + 1) * ixs],
                                  idx_d.ap()[:, b * ixs:(b + 1) * ixs])
                hT = hT_pool.tile([128, mlp_block], BF16)
                for t0 in range(0, mlp_block, stripe):
                    ps1 = ps1_pool.tile([128, stripe], F32)
                    for j in range(nkb):
                        nc.tensor.matmul(ps1[:], w1_t[:, j, :],
                                         dn[:, j, t0:t0 + stripe],
                                         start=(j == 0), stop=(j == nkb - 1))
                    nc.scalar.activation(hT[:, t0:t0 + stripe], ps1[:], relu,
                                         bias=b1_t[:])
                for s in range(spb):
                    ps2 = ps2_pool.tile([128, d_out], F32)
                    nc.tensor.matmul(ps2[:], hT[:, s * 128:(s + 1) * 128],
                                     w2s_t[:], start=True, stop=True)
                    nc.vector.tensor_copy(h2sb[:, b * spb + s, :], ps2[:])
                nc.sync.dma_start(h2x_blk[b],
                                  h2sb[:, b * spb:(b + 1) * spb, :])

        # ---------------- Phase B: gather + combine ----------------
        cme_q = 0
        with tc.tile_pool(name="gat", bufs=10) as g_pool, \
             tc.tile_pool(name="upt", bufs=4) as up_pool, \
             tc.tile_pool(name="tt", bufs=4) as t_pool, \
             tc.tile_pool(name="ot", bufs=4) as o_pool:
            for c in range(nchunk):
                sl = ix_all[:, c * (ni // 16):(c + 1) * (ni // 16)]
                g = g_pool.tile([128, K * chunk], BF16)
                upt = up_pool.tile([128, chunk], BF16)
                t = t_pool.tile([128, chunk], F32)
                o = o_pool.tile([128, chunk], BF16)
                if is_tr[c]:
                    # feature-major SBUF-source transpose gather, queue 3
                    nc.gpsimd.dma_gather(
                        g[:].rearrange("p (o n) -> p o n", o=1), h2sb[:], sl,
                        num_idxs=ni, num_idxs_reg=ni, elem_size=d_out,
                        transpose=True, single_packet=False, queue_num=3,
                        sbuf_tokens_per_rank=128,
                        sbuf_free_dim_per_rank=d_out * 2,
                        sbuf_free_dim_pad_per_rank=0, sbuf_byte_offset=0)
                    nc.sync.dma_start(upt[:], uptr_d.ap()[tr_ix[c]])
                    nc.vector.tensor_add(t[:], g[:, 0:chunk],
                                         g[:, chunk:2 * chunk])
                    nc.vector.tensor_add(t[:], t[:],
                                         g[:, 2 * chunk:3 * chunk])
                    nc.vector.tensor_add(o[:], t[:], upt[:])
                    nc.sync.dma_start(outt_d.ap()[tr_ix[c]], o[:])
                else:
                    # token-major CME gather from DRAM, queues 0..2
                    gv = g[:].rearrange("p (k u f) -> p (k u) f", k=K, u=upc)
                    nc.gpsimd.dma_gather(
                        gv, h2x_d.ap(), sl,
                        num_idxs=ni, num_idxs_reg=ni, elem_size=d_out,
                        single_packet=False, queue_num=cme_q)
                    cme_q = (cme_q + 1) % 3
                    g4 = g[:].rearrange("p (k u f) -> p k (u f)", k=K, u=upc)
                    nc.sync.dma_start(
                        upt[:].rearrange("p (u f) -> p u f", u=upc),
                        upc_d.ap()[cme_ix[c]])
                    nc.vector.tensor_add(t[:], g4[:, 0], g4[:, 1])
                    nc.vector.tensor_add(t[:], t[:], g4[:, 2])
                    nc.vector.tensor_add(o[:], t[:], upt[:])
                    nc.sync.dma_start(
                        outc_d.ap()[cme_ix[c]],
                        o[:].rearrange("p (u f) -> p u f", u=upc))

    nc.compile()
    _BUILD_CACHE[key] = nc
    return nc


def _prep_core_inputs(down_features, up_features, idx, W1, b1, W2, b2, n,
                      ld=LD, lu=LU, d_in=D_IN, d_out=D_OUT, chunk=CHUNK):
    """Host-side packing of the full inputs into core n's input map."""
    nchunk = lu // chunk
    upc = chunk // 128
    ni = K * chunk
    is_tr, cme_ix, tr_ix, n_cme, n_tr = _chunk_types(nchunk)

    downt = np.ascontiguousarray(
        down_features[:, n, :].T).astype(ml_dtypes.bfloat16)
    upn = up_features[:, n, :].astype(np.float32) + b2[None, :].astype(np.float32)
    upn = upn.reshape(nchunk, chunk, d_out)
    idxn = idx[:, n, :].astype(np.int16).reshape(nchunk, chunk, K)

    upcme = np.zeros((max(n_cme, 1), 128, upc, d_out), np.float32)
    uptr = np.zeros((max(n_tr, 1), 128, chunk), np.float32)
    flat = np.empty((nchunk, ni), np.int16)
    for c in range(nchunk):
        if is_tr[c]:
            # slots j = k*chunk + i; feature-major up
            flat[c] = idxn[c].T.reshape(ni)                    # [k, i]
            uptr[tr_ix[c]] = upn[c].T
        else:
            # slots j = (k*upc + u)*128 + p; token-major chunk layout
            flat[c] = idxn[c].reshape(upc, 128, K).transpose(2, 0, 1).reshape(ni)
            upcme[cme_ix[c]] = upn[c].reshape(upc, 128, d_out).transpose(1, 0, 2)

    wrapped = flat.reshape(nchunk, ni // 16, 16).transpose(0, 2, 1)
    rep = np.tile(wrapped, (1, 8, 1))                          # [c, 128, ni/16]
    idxp = np.ascontiguousarray(
        rep.transpose(1, 0, 2).reshape(128, nchunk * (ni // 16)))

    return {
        "downt": downt,
        "w1": np.ascontiguousarray(W1).astype(ml_dtypes.bfloat16),
        "b1": np.ascontiguousarray(b1.astype(np.float32).reshape(d_out, 1)),
        "w2s": np.ascontiguousarray(W2.astype(np.float32)
                                    / np.float32(K)).astype(ml_dtypes.bfloat16),
        "upcme": upcme.astype(ml_dtypes.bfloat16),
        "uptr": uptr.astype(ml_dtypes.bfloat16),
        "idxp": idxp,
    }


def _unpack_out(res_map, lu=LU, d_out=D_OUT, chunk=CHUNK):
    nchunk = lu // chunk
    upc = chunk // 128
    is_tr, cme_ix, tr_ix, n_cme, n_tr = _chunk_types(nchunk)
    outc = np.asarray(res_map["outc"]).astype(np.float32)
    outt = np.asarray(res_map["outt"]).astype(np.float32)
    out = np.empty((lu, d_out), np.float32)
    for c in range(nchunk):
        dst = out[c * chunk:(c + 1) * chunk]
        if is_tr[c]:
            dst[:] = outt[tr_ix[c]].T
        else:
            dst[:] = outc[cme_ix[c]].transpose(1, 0, 2).reshape(chunk, d_out)
    return out


def kernel(down_features, up_features, idx, W1, b1, W2, b2):
    down_features = np.asarray(down_features)
    up_features = np.asarray(up_features)
    idx = np.asarray(idx)
    W1, b1, W2, b2 = (np.asarray(a) for a in (W1, b1, W2, b2))

    nc = _build()
    in_maps = [
        _prep_core_inputs(down_features, up_features, idx, W1, b1, W2, b2, n)
        for n in range(NCORES)
    ]
    res = run_bass_kernel_spmd(nc, in_maps, core_ids=list(range(NCORES)))
    cols = [_unpack_out(res.results[n]) for n in range(NCORES)]
    return np.stack(cols, axis=1).astype(np.float32)


# revision 16
# speedup vs baseline: 1.2123x; 1.0070x over previous
"""Trainium2 Bass kernel for KNN-upsample (MLP on down points + KNN mean + residual).

Contract: kernel(**inputs) takes FULL numpy inputs (as produced by
setup_inputs) and returns the FULL (LU, N, D_OUT) float32 output.

Sharding: batch axis N=8 -> one NeuronCore per batch column (data
parallel, no cross-core communication).

Per-core device program:
  Phase A (MLP, bf16): h2[m, :] = relu(down[m, :] @ W1 + b1) @ (W2/3),
    computed per 512-token PSUM stripe into a persistent SBUF table
    h2sb[p, r, f] = h2[r*128+p, f] (bf16, 32 KB/partition), which is
    also staged out to a DRAM copy h2x (256 B rows).
  Phase B (gather + combine): the point space is cut into chunks of
    C=1024.  Three of every four chunks gather their K*C rows from the
    DRAM copy with the plain CME dma_gather on SWDGE queues 0..2 -
    descriptor generation runs on three Q7 core pairs concurrently.
    Every fourth chunk instead uses the transposing SBUF-source gather
    on queue 3 (the fourth core pair), which reads h2sb over the SBUF
    fabric and so takes no HBM random-read bandwidth.  The transpose
    stream is serialized with itself (one queue): overlapping transpose
    gathers corrupt each other through the shared XBAR, but one
    transpose stream + CME streams verify bit-exact.
    CME chunks combine token-major (slot j = (k*upc+u)*128+p), transpose
    chunks feature-major (slot j = k*C+i); DVE sums the three k-planes
    plus the bias-folded up features; results DMA out in bf16 and the
    host upcasts.

Host-side preprocessing (pure data-layout / index work):
  - down column transposed to (D_IN, LD), cast bf16; W1, W2/3 bf16
  - b2 folded into up; up packed per chunk type (token-major
    [cc, p, u, f] / feature-major [ct, f, i]) in bf16
  - idx cast int16, slot-ordered per chunk type, wrapped into the
    [16, ni/16] SWDGE layout, replicated x8, all chunks concatenated
  - outputs unpacked per chunk type and upcast to fp32
"""

import numpy as np
import ml_dtypes
from contextlib import ExitStack

import concourse.bacc as bacc
import concourse.tile as tile
import concourse.mybir as mybir
from concourse.bass_utils import run_bass_kernel_spmd

LD, LU, N, D_IN, D_OUT, K = 16384, 65536, 8, 256, 128, 3

CHUNK = 1024            # upsample points per gather chunk
MLP_BLOCK = 2048        # down points per MLP dma block
NCORES = 8
TR_EVERY = 4            # every TR_EVERY-th chunk uses the SBUF transpose path

F32 = mybir.dt.float32
BF16 = mybir.dt.bfloat16
I16 = mybir.dt.int16

_BUILD_CACHE = {}


def _chunk_types(nchunk):
    """Returns (is_tr list, cme order index, tr order index, counts)."""
    is_tr, cme_ix, tr_ix = [], [], []
    nc_, nt_ = 0, 0
    for c in range(nchunk):
        t = (c % TR_EVERY == TR_EVERY - 1)
        is_tr.append(t)
        cme_ix.append(None if t else nc_)
        tr_ix.append(nt_ if t else None)
        nc_ += 0 if t else 1
        nt_ += 1 if t else 0
    return is_tr, cme_ix, tr_ix, nc_, nt_


def _build(ld=LD, lu=LU, d_in=D_IN, d_out=D_OUT, chunk=CHUNK, mlp_block=MLP_BLOCK):
    """Build + compile the per-core Bass program (identical on all cores)."""
    key = (ld, lu, d_in, d_out, chunk, mlp_block)
    if key in _BUILD_CACHE:
        return _BUILD_CACHE[key]

    nchunk = lu // chunk
    upc = chunk // 128                  # 128-point groups per chunk
    ni = K * chunk                      # gather indices per chunk
    nkb = d_in // 128                   # contraction tiles for matmul 1
    nblk = ld // mlp_block              # MLP dma blocks
    spb = mlp_block // 128              # 128-token sub-tiles per MLP block
    nrank = ld // 128

    is_tr, cme_ix, tr_ix, n_cme, n_tr = _chunk_types(nchunk)

    nc = bacc.Bacc("TRN2", target_bir_lowering=False, debug=False,
                   num_swdge_queues=4)

    downt_d = nc.dram_tensor("downt", (d_in, ld), BF16, kind="ExternalInput")
    w1_d = nc.dram_tensor("w1", (d_in, d_out), BF16, kind="ExternalInput")
    b1_d = nc.dram_tensor("b1", (d_out, 1), F32, kind="ExternalInput")
    w2s_d = nc.dram_tensor("w2s", (d_out, d_out), BF16, kind="ExternalInput")
    upc_d = nc.dram_tensor("upcme", (max(n_cme, 1), 128, upc, d_out), BF16,
                           kind="ExternalInput")
    uptr_d = nc.dram_tensor("uptr", (max(n_tr, 1), 128, chunk), BF16,
                            kind="ExternalInput")
    idx_d = nc.dram_tensor("idxp", (128, nchunk * (ni // 16)), I16,
                           kind="ExternalInput")
    outc_d = nc.dram_tensor("outc", (max(n_cme, 1), 128, upc, d_out), BF16,
                            kind="ExternalOutput")
    outt_d = nc.dram_tensor("outt", (max(n_tr, 1), 128, chunk), BF16,
                            kind="ExternalOutput")
    h2x_d = nc.dram_tensor("h2x", (ld, d_out), BF16, kind="Internal")

    relu = mybir.ActivationFunctionType.Relu

    # h2x viewed so a whole MLP block stores with one DMA:
    # row index m = b*mlp_block + s*128 + p  ->  [b, p, s, f]
    h2x_blk = h2x_d.ap().rearrange("(b s p) f -> b p s f", b=nblk, s=spb, p=128)

    with tile.TileContext(nc) as tc, ExitStack() as ctx:
        consts = ctx.enter_context(tc.tile_pool(name="consts", bufs=1))
        ix_all = consts.tile([128, nchunk * (ni // 16)], I16)
        nc.sync.dma_start(ix_all[:], idx_d.ap())
        w1_t = consts.tile([128, nkb, d_out], BF16)
        w1_v = w1_d.ap().rearrange("(j p) e -> j p e", j=nkb, p=128)
        for j in range(nkb):
            nc.sync.dma_start(w1_t[:, j, :], w1_v[j])
        w2s_t = consts.tile([128, d_out], BF16)
        nc.sync.dma_start(w2s_t[:], w2s_d.ap())
        b1_t = consts.tile([128, 1], F32)
        nc.sync.dma_start(b1_t[:], b1_d.ap())
        # persistent token-major h2 table: [p, r, f] = h2[r*128+p, f]
        h2sb = consts.tile([128, nrank, d_out], BF16)

        # ---------------- Phase A: MLP ----------------
        stripe = 512                      # PSUM bank / max moving free dim
        with tc.tile_pool(name="dn", bufs=4) as dn_pool, \
             tc.tile_pool(name="ps1", bufs=3, space="PSUM") as ps1_pool, \
             tc.tile_pool(name="hT", bufs=3) as hT_pool, \
             tc.tile_pool(name="ps2", bufs=4, space="PSUM") as ps2_pool:
            for b in range(nblk):
                dn = dn_pool.tile([128, nkb, mlp_block], BF16)
                for j in range(nkb):
                    nc.sync.dma_start(
                        dn[:, j, :],
                        downt_d.ap()[j * 128:(j + 1) * 128,
                                     b * mlp_block:(b + 1) * mlp_block])
                hT = hT_pool.tile([128, mlp_block], BF16)
                for t0 in range(0, mlp_block, stripe):
                    ps1 = ps1_pool.tile([128, stripe], F32)
                    for j in range(nkb):
                        nc.tensor.matmul(ps1[:], w1_t[:, j, :],
                                         dn[:, j, t0:t0 + stripe],
                                         start=(j == 0), stop=(j == nkb - 1))
                    nc.scalar.activation(hT[:, t0:t0 + stripe], ps1[:], relu,
                                         bias=b1_t[:])
                for s in range(spb):
                    ps2 = ps2_pool.tile([128, d_out], F32)
                    nc.tensor.matmul(ps2[:], hT[:, s * 128:(s + 1) * 128],
                                     w2s_t[:], start=True, stop=True)
                    nc.vector.tensor_copy(h2sb[:, b * spb + s, :], ps2[:])
                nc.sync.dma_start(h2x_blk[b],
                                  h2sb[:, b * spb:(b + 1) * spb, :])

        # ---------------- Phase B: gather + combine ----------------
        cme_q = 0
        with tc.tile_pool(name="gat", bufs=10) as g_pool, \
             tc.tile_pool(name="upt", bufs=4) as up_pool, \
             tc.tile_pool(name="tt", bufs=4) as t_pool, \
             tc.tile_pool(name="ot", bufs=4) as o_pool:
            for c in range(nchunk):
                sl = ix_all[:, c * (ni // 16):(c + 1) * (ni // 16)]
                g = g_pool.tile([128, K * chunk], BF16)
                upt = up_pool.tile([128, chunk], BF16)
                t = t_pool.tile([128, chunk], F32)
                o = o_pool.tile([128, chunk], BF16)
                if is_tr[c]:
                    # feature-major SBUF-source transpose gather, queue 3
                    nc.gpsimd.dma_gather(
                        g[:].rearrange("p (o n) -> p o n", o=1), h2sb[:], sl,
                        num_idxs=ni, num_idxs_reg=ni, elem_size=d_out,
                        transpose=True, single_packet=False, queue_num=3,
                        sbuf_tokens_per_rank=128,
                        sbuf_free_dim_per_rank=d_out * 2,
                        sbuf_free_dim_pad_per_rank=0, sbuf_byte_offset=0)
                    nc.sync.dma_start(upt[:], uptr_d.ap()[tr_ix[c]])
                    nc.vector.tensor_add(t[:], g[:, 0:chunk],
                                         g[:, chunk:2 * chunk])
                    nc.vector.tensor_add(t[:], t[:],
                                         g[:, 2 * chunk:3 * chunk])
                    nc.vector.tensor_add(o[:], t[:], upt[:])
                    nc.sync.dma_start(outt_d.ap()[tr_ix[c]], o[:])
                else:
                    # token-major CME gather from DRAM, queues 0..2
                    gv = g[:].rearrange("p (k u f) -> p (k u) f", k=K, u=upc)
                    nc.gpsimd.dma_gather(
                        gv, h2x_d.ap(), sl,
                        num_idxs=ni, num_idxs_reg=ni, elem_size=d_out,
                        single_packet=False, queue_num=cme_q)
                    cme_q = (cme_q + 1) % 3
                    g4 = g[:].rearrange("p (k u f) -> p k (u f)", k=K, u=upc)
                    nc.sync.dma_start(
                        upt[:].rearrange("p (u f) -> p u f", u=upc),
                        upc_d.ap()[cme_ix[c]])
                    nc.vector.tensor_add(t[:], g4[:, 0], g4[:, 1])
                    nc.vector.tensor_add(t[:], t[:], g4[:, 2])
                    nc.vector.tensor_add(o[:], t[:], upt[:])
                    nc.sync.dma_start(
                        outc_d.ap()[cme_ix[c]],
                        o[:].rearrange("p (u f) -> p u f", u=upc))

    nc.compile()
    _BUILD_CACHE[key] = nc
    return nc


def _prep_core_inputs(down_features, up_features, idx, W1, b1, W2, b2, n,
                      ld=LD, lu=LU, d_in=D_IN, d_out=D_OUT, chunk=CHUNK):
    """Host-side packing of the full inputs into core n's input map."""
    nchunk = lu // chunk
    upc = chunk // 128
    ni = K * chunk
    is_tr, cme_ix, tr_ix, n_cme, n_tr = _chunk_types(nchunk)

    downt = np.ascontiguousarray(
        down_features[:, n, :].T).astype(ml_dtypes.bfloat16)
    upn = up_features[:, n, :].astype(np.float32) + b2[None, :].astype(np.float32)
    upn = upn.reshape(nchunk, chunk, d_out)
    idxn = idx[:, n, :].astype(np.int16).reshape(nchunk, chunk, K)

    upcme = np.zeros((max(n_cme, 1), 128, upc, d_out), np.float32)
    uptr = np.zeros((max(n_tr, 1), 128, chunk), np.float32)
    flat = np.empty((nchunk, ni), np.int16)
    for c in range(nchunk):
        if is_tr[c]:
            # slots j = k*chunk + i; feature-major up
            flat[c] = idxn[c].T.reshape(ni)                    # [k, i]
            uptr[tr_ix[c]] = upn[c].T
        else:
            # slots j = (k*upc + u)*128 + p; token-major chunk layout
            flat[c] = idxn[c].reshape(upc, 128, K).transpose(2, 0, 1).reshape(ni)
            upcme[cme_ix[c]] = upn[c].reshape(upc, 128, d_out).transpose(1, 0, 2)

    wrapped = flat.reshape(nchunk, ni // 16, 16).transpose(0, 2, 1)
    rep = np.tile(wrapped, (1, 8, 1))                          # [c, 128, ni/16]
    idxp = np.ascontiguousarray(
        rep.transpose(1, 0, 2).reshape(128, nchunk * (ni // 16)))

    return {
        "downt": downt,
        "w1": np.ascontiguousarray(W1).astype(ml_dtypes.bfloat16),
        "b1": np.ascontiguousarray(b1.astype(np.float32).reshape(d_out, 1)),
        "w2s": np.ascontiguousarray(W2.astype(np.float32)
                                    / np.float32(K)).astype(ml_dtypes.bfloat16),
        "upcme": upcme.astype(ml_dtypes.bfloat16),
        "uptr": uptr.astype(ml_dtypes.bfloat16),
        "idxp": idxp,
    }


def _unpack_out(res_map, lu=LU, d_out=D_OUT, chunk=CHUNK):
    nchunk = lu // chunk
    upc = chunk // 128
    is_tr, cme_ix, tr_ix, n_cme, n_tr = _chunk_types(nchunk)
    outc = np.asarray(res_map["outc"]).astype(np.float32)
    outt = np.asarray(res_map["outt"]).astype(np.float32)
    out = np.empty((lu, d_out), np.float32)
    for c in range(nchunk):
        dst = out[c * chunk:(c + 1) * chunk]
        if is_tr[c]:
            dst[:] = outt[tr_ix[c]].T
        else:
            dst[:] = outc[cme_ix[c]].transpose(1, 0, 2).reshape(chunk, d_out)
    return out


def kernel(down_features, up_features, idx, W1, b1, W2, b2):
    down_features = np.asarray(down_features)
    up_features = np.asarray(up_features)
    idx = np.asarray(idx)
    W1, b1, W2, b2 = (np.asarray(a) for a in (W1, b1, W2, b2))

    nc = _build()
    in_maps = [
        _prep_core_inputs(down_features, up_features, idx, W1, b1, W2, b2, n)
        for n in range(NCORES)
    ]
    res = run_bass_kernel_spmd(nc, in_maps, core_ids=list(range(NCORES)))
    cols = [_unpack_out(res.results[n]) for n in range(NCORES)]
    return np.stack(cols, axis=1).astype(np.float32)
